# revision 4
# baseline (speedup 1.0000x reference)
"""Causal multi-head attention (B=1, S=4096, D=1024, 16 heads) on 8 TRN2
NeuronCores, head-sharded tensor-parallel: 2 heads per core, partial output
projections summed on the host.

Numerics: bf16 PE datapath everywhere (PSUM accumulation fp32); softmax
scale (1/8) folded into the exp activation; exact-divide reciprocal of the
softmax denominators (accumulated for free as a ones-column in the AV
stationary); causal masking by trimming matmul columns to the causal
frontier plus one 128x128 0/1 triangle multiply (Pool engine) per diagonal
block. End-to-end rel err ~5e-3 vs the fp32 reference.

Scheduling: per q-block k-loop of scores (PE) -> exp (Act, from PSUM) ->
AV (PE, deferred 2 blocks so the PE never waits on the Act engine); filler
work (next s-block's QKV projection, previous q-block's output projection)
split into ~1us pieces distributed evenly across the loop; four pieces held
back for each q-block boundary to hide the reciprocal chain; projection
outputs drained as bf16 partials (halves outbound DMA).
"""
import sys
sys.path.insert(0, '/opt/trn_rl_repo')
import numpy as np
import concourse.bacc as bacc
import concourse.tile as tile
from concourse import mybir

BF16 = mybir.dt.bfloat16
F32 = mybir.dt.float32

S = 4096          # sequence length
D = 1024          # d_model
P = 128           # partitions / per-core feature width (2 heads x 64)
NSB = S // 512    # 8 s-blocks of 512
NKC = D // P      # 8 contraction chunks for projections
SCALE = 0.125     # 1/sqrt(64)


def _alloc_pools(tc):
    all_pools = []

    def pool(**kw):
        p = tc.alloc_tile_pool(**kw)
        all_pools.append(p)
        return p

    pools = {
        "consts": pool(name="consts", bufs=1),
        "big": pool(name="big", bufs=1),
        "xs_pool": pool(name="xs", bufs=2),
        "es_pool": pool(name="es", bufs=4),
        "at_pool": pool(name="at", bufs=2),
        "nrm_pool": pool(name="nrm", bufs=2),
        "po_pool": pool(name="po", bufs=3),
        "ps_sc": pool(name="ps_sc", bufs=2, space="PSUM"),
        "ps_av": pool(name="ps_av", bufs=2, space="PSUM"),
        "ps_qkv": pool(name="ps_qkv", bufs=2, space="PSUM"),
    }
    return pools, all_pools


def _emit_body(tc, nc, ap, pools):
    consts = pools["consts"]
    big = pools["big"]
    xs_pool = pools["xs_pool"]
    es_pool = pools["es_pool"]
    at_pool = pools["at_pool"]
    nrm_pool = pools["nrm_pool"]
    po_pool = pools["po_pool"]
    ps_sc = pools["ps_sc"]
    ps_av = pools["ps_av"]
    ps_qkv = pools["ps_qkv"]

    w_q = consts.tile([P, NKC, P], BF16, tag="w_q")
    w_k = consts.tile([P, NKC, P], BF16, tag="w_k")
    w_v = consts.tile([P, NKC, P], BF16, tag="w_v")
    w_o = consts.tile([P, D], BF16, tag="w_o")
    maskt = consts.tile([P, P], BF16, tag="mask")
    ones64 = consts.tile([1, 64], BF16, tag="ones64")
    nc.vector.memset(ones64, 1.0)
    wq_r = ap["wq"].rearrange("(c p) m -> p c m", p=P)
    nc.sync.dma_start(out=w_q[:, 0:4, :], in_=wq_r[:, 0:4, :])

    def load_mid_consts():
        nc.sync.dma_start(out=w_q[:, 4:8, :], in_=wq_r[:, 4:8, :])
        nc.sync.dma_start(out=w_k, in_=ap["wk"].rearrange("(c p) m -> p c m", p=P))
        nc.sync.dma_start(out=w_v, in_=ap["wv"].rearrange("(c p) m -> p c m", p=P))
        nc.sync.dma_start(out=maskt, in_=ap["mask"])

    def load_late_consts():
        nc.sync.dma_start(out=w_o, in_=ap["wo"])

    qt = [big.tile([P, 512], BF16, tag=f"qt{i}", name=f"qt{i}") for i in range(NSB)]
    kt = [big.tile([P, 512], BF16, tag=f"kt{i}", name=f"kt{i}") for i in range(NSB)]
    # V natural per 128-k-block: [v_h0 (64) | 1 | v_h1 (64) | 1]
    vnat = [big.tile([P, 4, 130], BF16, tag=f"vn{i}", name=f"vn{i}") for i in range(NSB)]
    for i in range(NSB):
        nc.vector.memset(vnat[i], 1.0)

    xT_r = ap["xT"].rearrange("(c p) s -> p c s", p=P)
    outT_r = ap["outT"].rearrange("(c p) s -> p c s", p=P)

    def emit_qkv_pieces(sb):
        """QKV projection for s-block sb as a list of ~1us filler pieces."""
        xs = xs_pool.tile([P, NKC, 512], BF16, tag="xs")
        state = {}

        def dma_in():
            if sb == 0:
                nc.sync.dma_start(out=xs[:, 0:4, :],
                                  in_=xT_r[:, 0:4, 0:512])
                load_mid_consts()
                nc.sync.dma_start(out=xs[:, 4:8, :],
                                  in_=xT_r[:, 4:8, 0:512])
            else:
                nc.sync.dma_start(out=xs,
                                  in_=xT_r[:, :, sb * 512:(sb + 1) * 512])

        def proj_half(wt, half, ps_key):
            if half == 0:
                state[ps_key] = ps_qkv.tile([P, 512], F32, tag="qkv",
                                            name="ps_qk")
            ps = state[ps_key]
            for kc in range(4 * half, 4 * half + 4):
                nc.tensor.matmul(ps, lhsT=wt[:, kc, :], rhs=xs[:, kc, :],
                                 start=(kc == 0), stop=(kc == NKC - 1))

        def v_half(half):
            if half == 0:
                state["v"] = ps_qkv.tile([P, 512], F32, tag="qkv", name="ps_v")
            ps2 = state["v"]
            for t in (2 * half, 2 * half + 1):
                for kc in range(NKC):
                    nc.tensor.matmul(ps2[:, t * P:(t + 1) * P],
                                     lhsT=xs[:, kc, t * P:(t + 1) * P],
                                     rhs=w_v[:, kc, :],
                                     start=(kc == 0), stop=(kc == NKC - 1))

        def v_done():
            ps2 = state["v"]
            nc.vector.tensor_copy(
                out=vnat[sb][:, :, 0:64],
                in_=ps2.rearrange("p (t f) -> p t f", t=4)[:, :, 0:64])
            nc.vector.tensor_copy(
                out=vnat[sb][:, :, 65:129],
                in_=ps2.rearrange("p (t f) -> p t f", t=4)[:, :, 64:128])

        return [
            dma_in,
            lambda: proj_half(w_q, 0, "q"),
            lambda: (proj_half(w_q, 1, "q"),
                     nc.vector.tensor_copy(out=qt[sb], in_=state["q"])),
            lambda: proj_half(w_k, 0, "k"),
            lambda: (proj_half(w_k, 1, "k"),
                     nc.vector.tensor_copy(out=kt[sb], in_=state["k"])),
            lambda: v_half(0),
            lambda: (v_half(1), v_done()),
        ]

    def emit_attention(qb, work, held):
        """Score/exp/AV loop for q-block qb. `work` pieces are spread across
        the k-loop; `held` pieces run between the last AV and the normalize
        broadcast (hiding the reciprocal chain latency). Returns the
        projection pieces for the NEXT q-block's filler."""
        nkb = 4 * (qb + 1)
        av0 = ps_av.tile([65, 512], F32, tag="av")
        av1 = ps_av.tile([65, 512], F32, tag="av")
        pending_av = []
        done = 0
        for kb in range(nkb):
            sb, t = kb // 4, kb % 4
            j = kb - 4 * qb                 # >= 0 on diagonal blocks
            lo = 128 * j if j > 0 else 0    # first live (unmasked) column
            sc = ps_sc.tile([P, 1024], F32, tag="sc")
            # scores for both heads (separate 64-row contractions)
            nc.tensor.matmul(sc[:, lo:512],
                             lhsT=kt[sb][0:64, t * P:(t + 1) * P],
                             rhs=qt[qb][0:64, lo:512], start=True, stop=True)
            nc.tensor.matmul(sc[:, 512 + lo:1024],
                             lhsT=kt[sb][64:128, t * P:(t + 1) * P],
                             rhs=qt[qb][64:128, lo:512], start=True, stop=True)
            es = es_pool.tile([P, 1024], BF16, tag="es")
            if lo == 0:
                nc.scalar.activation(out=es, in_=sc,
                                     func=mybir.ActivationFunctionType.Exp,
                                     scale=SCALE)
            else:
                # one strided activation covering both heads' live columns
                sc2 = sc.rearrange("p (h c) -> p h c", h=2)
                es2 = es.rearrange("p (h c) -> p h c", h=2)
                nc.scalar.activation(out=es2[:, :, lo:512],
                                     in_=sc2[:, :, lo:512],
                                     func=mybir.ActivationFunctionType.Exp,
                                     scale=SCALE)
            if j >= 0:
                # only the [128, 128] triangle at the causal frontier is
                # masked (DVE: gpsimd dispatch overhead is unmodeled on HW)
                nc.vector.tensor_tensor(out=es[:, lo:lo + 128],
                                        in0=es[:, lo:lo + 128],
                                        in1=maskt,
                                        op=mybir.AluOpType.mult)
                nc.vector.tensor_tensor(out=es[:, 512 + lo:512 + lo + 128],
                                        in0=es[:, 512 + lo:512 + lo + 128],
                                        in1=maskt,
                                        op=mybir.AluOpType.mult)
            first, last = (kb == 0), (kb == nkb - 1)

            # software-pipeline: this block's AV is emitted after the NEXT
            # block's scores so the PE never waits on this block's exp
            def emit_av(sb=sb, t=t, lo=lo, es=es, first=first, last=last):
                nc.tensor.matmul(av0[:, lo:512], lhsT=vnat[sb][:, t, 0:65],
                                 rhs=es[:, lo:512], start=first, stop=last,
                                 skip_group_check=True)
                nc.tensor.matmul(av1[:, lo:512], lhsT=vnat[sb][:, t, 65:130],
                                 rhs=es[:, 512 + lo:1024], start=first,
                                 stop=last, skip_group_check=True)
            if len(pending_av) >= 2:    # 2-deep software pipeline
                pending_av.pop(0)()
            pending_av.append(emit_av)
            # distribute filler pieces evenly over the k-loop
            target = ((kb + 1) * len(work)) // nkb
            while done < target:
                work[done]()
                done += 1
        for av in pending_av:
            av()
        # normalize: exact reciprocal of the denominator rows (partition 64),
        # split per head so head0's broadcast can start before head1's recip
        rf = nrm_pool.tile([1, 2, 512], F32, tag="rf")
        rb = nrm_pool.tile([1, 2, 512], BF16, tag="rb")
        nc.vector.reciprocal(out=rf[:, 0, :], in_=av0[64:65, :])
        nc.scalar.copy(out=rb[:, 0, :], in_=rf[:, 0, :])
        nc.vector.reciprocal(out=rf[:, 1, :], in_=av1[64:65, :])
        nc.scalar.copy(out=rb[:, 1, :], in_=rf[:, 1, :])
        for piece in held:          # PE-busy filler while the DVE chain runs
            piece()
        bc = nrm_pool.tile([P, 512], BF16, tag="bc")
        for i, lo in ((0, 0), (1, 64)):
            bb = ps_qkv.tile([64, 512], F32, tag="qkv", name="bb")
            nc.tensor.matmul(bb, lhsT=ones64, rhs=rb[:, i, :], start=True,
                             stop=True)
            nc.vector.tensor_copy(out=bc[lo:lo + 64, :], in_=bb)
        at = at_pool.tile([P, 512], BF16, tag="at")
        nc.vector.tensor_tensor(out=at[0:64, :], in0=av0[0:64, :],
                                in1=bc[0:64, :], op=mybir.AluOpType.mult)
        nc.vector.tensor_tensor(out=at[64:128, :], in0=av1[0:64, :],
                                in1=bc[64:128, :], op=mybir.AluOpType.mult)

        def proj_piece(mc, qb=qb, at=at):
            def run():
                pp = ps_qkv.tile([P, 512], F32, tag="qkv", name="pp")
                nc.tensor.matmul(pp, lhsT=w_o[:, mc * P:(mc + 1) * P], rhs=at,
                                 start=True, stop=True)
                po = po_pool.tile([P, 512], BF16, tag="po")
                if qb == NSB - 1 and mc % 2 == 1:
                    nc.scalar.copy(out=po, in_=pp)   # Act is idle in the tail
                else:
                    nc.vector.tensor_copy(out=po, in_=pp)
                nc.sync.dma_start(out=outT_r[:, mc, qb * 512:(qb + 1) * 512],
                                  in_=po)
            return run

        return [proj_piece(mc) for mc in range(NKC)]

    # schedule: pipeline proj(qb-1) and QKV(sb+1) into attention(qb)'s k-loop
    for piece in emit_qkv_pieces(0):
        piece()
    load_late_consts()
    pending_proj = []
    for qb in range(NSB):
        work = list(pending_proj)
        if qb + 1 < NSB:
            work.extend(emit_qkv_pieces(qb + 1))
        held = work[-4:]            # PE-busy filler for the boundary
        work = work[:-4]
        pending_proj = emit_attention(qb, work, held)
    for piece in pending_proj:
        piece()


def build(k_repeat=1):
    nc = bacc.Bacc("TRN2", target_bir_lowering=False, debug=False,
                   enable_asserts=False)
    ap = {}
    ap["xT"] = nc.dram_tensor("xT", [D, S], BF16, kind="ExternalInput").ap()
    ap["wq"] = nc.dram_tensor("wq", [D, P], BF16, kind="ExternalInput").ap()
    ap["wk"] = nc.dram_tensor("wk", [D, P], BF16, kind="ExternalInput").ap()
    ap["wv"] = nc.dram_tensor("wv", [D, P], BF16, kind="ExternalInput").ap()
    ap["wo"] = nc.dram_tensor("wo", [P, D], BF16, kind="ExternalInput").ap()
    ap["mask"] = nc.dram_tensor("mask", [P, P], BF16, kind="ExternalInput").ap()
    ap["outT"] = nc.dram_tensor("outT", [D, S], BF16, kind="ExternalOutput").ap()
    with tile.TileContext(nc) as tc, \
         nc.allow_low_precision(reason="bf16 PE path; accumulation stays fp32"):
        pools, all_pools = _alloc_pools(tc)
        if k_repeat == 1:
            _emit_body(tc, nc, ap, pools)
        else:
            # pools live outside the loop: no per-iteration drain, so the
            # next iteration's QKV overlaps the previous projection tail
            with tc.For_i(0, k_repeat, 1,
                          hint_engines=(mybir.EngineType.PE,
                                        mybir.EngineType.Activation,
                                        mybir.EngineType.DVE,
                                        mybir.EngineType.SP,
                                        mybir.EngineType.Pool)):
                _emit_body(tc, nc, ap, pools)
        for p in reversed(all_pools):
            p.release()
    nc.compile()
    return nc


def make_in_maps(x, Wq, Wk, Wv, Wo):
    """x [1,S,D] fp32 -> list of 8 per-core input dicts (bf16)."""
    import ml_dtypes
    bf = ml_dtypes.bfloat16
    xT = np.ascontiguousarray(np.asarray(x, dtype=np.float32)[0].T.astype(bf))
    ki = np.arange(P)[:, None]
    qi = np.arange(P)[None, :]
    mask = (qi >= ki).astype(bf)               # causal triangle, [128, 128]
    in_maps = []
    for c in range(8):
        cs = slice(c * P, (c + 1) * P)
        in_maps.append({
            "xT": xT,
            "wq": np.ascontiguousarray(np.asarray(Wq, np.float32)[:, cs].astype(bf)),
            "wk": np.ascontiguousarray(np.asarray(Wk, np.float32)[:, cs].astype(bf)),
            "wv": np.ascontiguousarray(np.asarray(Wv, np.float32)[:, cs].astype(bf)),
            "wo": np.ascontiguousarray(np.asarray(Wo, np.float32)[cs, :].astype(bf)),
            "mask": mask,
        })
    return in_maps


def combine(results):
    """Sum 8 partial outT [D, S] bf16 tensors and restore [1, S, D] fp32."""
    acc = np.zeros((D, S), dtype=np.float32)
    for r in results:
        acc += np.asarray(r["outT"]).astype(np.float32)
    return np.ascontiguousarray(acc.T)[None, :, :].astype(np.float32)


_NC_CACHE = {}


def kernel(x, Wq, Wk, Wv, Wo):
    from concourse import bass_utils
    if "nc" not in _NC_CACHE:
        _NC_CACHE["nc"] = build(k_repeat=1)
    nc = _NC_CACHE["nc"]
    in_maps = make_in_maps(x, Wq, Wk, Wv, Wo)
    res = bass_utils.run_bass_kernel_spmd(nc, in_maps, core_ids=list(range(8)))
    return combine(res.results)
